# revision 8
# baseline (speedup 1.0000x reference)
"""HebbianConv2d Trainium2 kernel.

Full-input contract: kernel(x, weight, bias) -> (y, delta_w)
  x      [32, 128, 64, 64] f32
  weight [256, 128, 3, 3]  f32
  bias   [256]             f32
  y      [32, 256, 62, 62] f32
  delta_w[256, 128, 3, 3]  f32

Sharding: data-parallel over batch, 4 images per core on 8 cores.
Each core computes its y shard plus a partial Hebbian sum; the host
concatenates y and sums/scales the partials (the WTA denominator is
exactly B*Ho*Wo since the argmax one-hot always sums to 1 per site).

Per-core pipeline (all fp32 except the Hebbian contraction in fp16):
  conv:  9 shifted-window taps over a padded 62x64 output grid,
         contraction over C=128 on partitions, PSUM-accumulated.
  mask:  PE-transpose y into [l', o] tiles (126-stride windows),
         reduce_max + fused is_equal*pattern -> one-hot fp16 mask.
  hebb:  per l'-tile, 9 DMA-transposed shifted copies of fp16 x;
         matmul lhsT=mask rhs=xT accumulating into one pinned PSUM
         tensor across all images; single drain at the end.
"""

import numpy as np

import concourse.bass as bass
import concourse.bacc as bacc
import concourse.mybir as mybir
import concourse.tile as tile
from concourse.bass_utils import run_bass_kernel_spmd

dt = mybir.dt

B, C, O, KS, H, W = 32, 128, 256, 3, 64, 64
NCORES = 8
BL = B // NCORES            # images per core
HO = WO = H - KS + 1        # 62
LP = HO * W                 # 3968: padded output grid (62 rows x 64 cols)
TW = 126                    # l' rows per hebb tile
NT = 32                     # hebb tiles (32*126 = 4032 >= 3968)
NCH = 8                     # conv N-chunks
CHW = LP // NCH             # 496
OT = 2                      # o tiles of 128
TAPS = KS * KS              # 9
XCOLS = 4224                # x sbuf cols (max read 126*31+130+128 = 4164)
YCOLS = 4040                # y sbuf cols (max read 126*31+128 = 4034)
HEBBC = OT * TAPS * C       # 2304 columns of the hebb psum accumulator
DEN = np.float32(B * HO * WO)  # 123008; +1e-8 is a no-op in fp32

_CACHE = {}


def _host_constants():
    ident = np.eye(128, dtype=np.float32)
    patt = np.zeros((128, NT), dtype=np.float32)
    for t in range(NT):
        for p in range(TW):
            lp = t * TW + p
            if lp < LP and (lp % W) < WO:
                patt[p, t] = 1.0
    return ident, patt


def build_nc():
    """Build and compile the per-core Bass program (SPMD, same on all cores)."""
    nc = bacc.Bacc("TRN2", target_bir_lowering=False, debug=False,
                   num_devices=NCORES)

    x_d = nc.dram_tensor("x", [BL, C, H, W], dt.float32, kind="ExternalInput").ap()
    w_d = nc.dram_tensor("weight", [O, C, KS, KS], dt.float32, kind="ExternalInput").ap()
    b_d = nc.dram_tensor("bias", [O], dt.float32, kind="ExternalInput").ap()
    id_d = nc.dram_tensor("ident", [128, 128], dt.float32, kind="ExternalInput").ap()
    pt_d = nc.dram_tensor("patt", [128, NT], dt.float32, kind="ExternalInput").ap()
    y_d = nc.dram_tensor("y", [BL, O, HO, WO], dt.float32, kind="ExternalOutput").ap()
    hb_d = nc.dram_tensor("hebb", [128, HEBBC], dt.float32, kind="ExternalOutput").ap()

    with tile.TileContext(nc) as tc:
        _body(nc, tc, x_d, w_d, b_d, id_d, pt_d, y_d, hb_d)

    nc.compile()
    return nc


def _body(nc, tc, x_d, w_d, b_d, id_d, pt_d, y_d, hb_d):
    f32, f16 = dt.float32, dt.float16
    w2d = w_d.rearrange("o c kh kw -> o (c kh kw)")   # [256, 1152]
    CKK = C * TAPS                                    # 1152

    with (
        tc.tile_pool(name="const", bufs=1) as constp,
        tc.tile_pool(name="wprep", bufs=1) as wprepp,
        tc.tile_pool(name="xp", bufs=2) as xpool,
        tc.tile_pool(name="x16p", bufs=2) as x16pool,
        tc.tile_pool(name="yp", bufs=2) as ypool,
        tc.tile_pool(name="ytp", bufs=3) as ytpool,
        tc.tile_pool(name="mp", bufs=NT) as maskpool,
        tc.tile_pool(name="xtp", bufs=4) as xtpool,
        tc.tile_pool(name="zp", bufs=1) as zpool,
        tc.tile_pool(name="hdr", bufs=1) as hdrainp,
        tc.tile_pool(name="pb", bufs=3, space="PSUM") as pbank,
        tc.tile_pool(name="ph", bufs=1, space="PSUM") as phebb,
    ):
        # ---- constants ----
        ident = constp.tile([128, 128], f32, name="ident")
        nc.sync.dma_start(ident[:], id_d[:])
        patt = constp.tile([128, NT], f32, name="patt")
        nc.sync.dma_start(patt[:], pt_d[:])
        bias_sb = constp.tile([128, OT], f32, name="bias_sb")
        nc.sync.dma_start(bias_sb[:], b_d.rearrange("(t p) -> p t", p=128))

        # ---- weight normalization ----
        # w2 tiles [o=128, 1152]; nrm over free dim; wn in [c, tap, o] layout.
        w2 = wprepp.tile([128, OT, CKK], f32, name="w2")
        for ot in range(OT):
            nc.sync.dma_start(w2[:, ot, :], w2d[ot * 128:(ot + 1) * 128, :])
        wsq = wprepp.tile([128, CKK], f32, name="wsq")
        n2 = wprepp.tile([128, OT], f32, name="n2")
        for ot in range(OT):
            nc.vector.tensor_mul(wsq[:], w2[:, ot, :], w2[:, ot, :])
            nc.vector.reduce_sum(n2[:, ot:ot + 1], wsq[:], axis=mybir.AxisListType.X)
        sq = wprepp.tile([128, OT], f32, name="sq")
        nc.scalar.sqrt(sq[:], n2[:])
        r0 = wprepp.tile([128, OT], f32, name="r0")
        nc.vector.reciprocal(r0[:], sq[:])
        # one Newton step on rsqrt: r = r0 * (1.5 - 0.5 * n2 * r0^2)
        t1 = wprepp.tile([128, OT], f32, name="t1")
        nc.vector.tensor_mul(t1[:], r0[:], r0[:])
        nc.vector.tensor_mul(t1[:], t1[:], n2[:])
        t2 = wprepp.tile([128, OT], f32, name="t2")
        nc.vector.tensor_scalar(t2[:], t1[:], -0.5, 1.5,
                                op0=mybir.AluOpType.mult, op1=mybir.AluOpType.add)
        rq = wprepp.tile([128, OT], f32, name="rq")
        nc.vector.tensor_mul(rq[:], r0[:], t2[:])
        wn2 = wprepp.tile([128, OT, CKK], f32, name="wn2")
        for ot in range(OT):
            nc.vector.tensor_scalar_mul(wn2[:, ot, :], w2[:, ot, :], rq[:, ot:ot + 1])
        # transpose to conv lhsT layout wn[c, tap, o]
        wn = wprepp.tile([128, TAPS, O], f32, name="wn")
        wn2v = wn2[:].rearrange("p t (c k) -> p t c k", k=TAPS)  # [128,OT,C,TAPS]
        for ot in range(OT):
            for tap in range(TAPS):
                wtr = pbank.tile([128, 128], f32, name="wtr", tag="pb")
                nc.tensor.transpose(wtr[:], wn2v[:, ot, :, tap], ident[:])
                nc.vector.tensor_copy(wn[:, tap, ot * 128:(ot + 1) * 128], wtr[:])

        # ---- hebb accumulator (pinned all-kernel) ----
        # One start=True matmul per PSUM bank zero-fills the whole bank, so
        # exactly 5 whole-bank zero matmuls run first (their outputs overlap
        # every chain, which also gives the scheduler the ordering dep); all
        # real chains then accumulate with start=False.
        hebb_ps = phebb.tile([128, HEBBC], f32, name="hebb_ps")
        zblk = zpool.tile([128, 512], f16, name="zblk")
        nc.vector.memset(zblk[:], 0.0)
        for b0 in range(0, HEBBC, 512):
            n = min(512, HEBBC - b0)
            nc.tensor.matmul(hebb_ps[:, b0:b0 + n], zblk[0:TW, 0:128],
                             zblk[0:TW, 0:n], start=True, stop=False)

        for img in range(BL):
            # ---- load + cast x ----
            x_sb = xpool.tile([128, XCOLS], f32, name="x_sb", tag="x")
            nc.sync.dma_start(x_sb[:, 0:H * W], x_d[img].rearrange("c h w -> c (h w)"))
            nc.vector.memset(x_sb[:, H * W:XCOLS], 0.0)
            x16 = x16pool.tile([128, XCOLS], f16, name="x16", tag="x16")
            nc.vector.tensor_copy(x16[:], x_sb[:])

            # ---- conv: y[o, l'] over padded grid ----
            y_sb = [ypool.tile([128, YCOLS], f32, name=f"y_sb{ot}", tag=f"y{ot}")
                    for ot in range(OT)]
            for ot in range(OT):
                nc.vector.memset(y_sb[ot][:, LP:YCOLS], 0.0)
                for nch in range(NCH):
                    yps = pbank.tile([128, CHW], f32, name="yps", tag="pb")
                    for tap in range(TAPS):
                        off = (tap // KS) * W + (tap % KS) + nch * CHW
                        nc.tensor.matmul(
                            yps[:], wn[:, tap, ot * 128:(ot + 1) * 128],
                            x_sb[:, off:off + CHW],
                            start=(tap == 0), stop=(tap == TAPS - 1))
                    nc.vector.tensor_scalar_add(
                        y_sb[ot][:, nch * CHW:(nch + 1) * CHW], yps[:],
                        bias_sb[:, ot:ot + 1])
                # ---- store y shard ----
                ysrc = y_sb[ot][:, 0:LP].rearrange("p (h w) -> p h w", w=W)
                nc.sync.dma_start(y_d[img, ot * 128:(ot + 1) * 128, :, :],
                                  ysrc[:, :, 0:WO])

            # ---- WTA mask tiles [l'=126, o=256] ----
            masks = []
            for t in range(NT):
                yt = ytpool.tile([128, O], f32, name="yt", tag="yt")
                for ot in range(OT):
                    ytr = pbank.tile([128, 128], f32, name="ytr", tag="pb")
                    nc.tensor.transpose(ytr[:], y_sb[ot][:, t * TW:t * TW + 128],
                                        ident[:])
                    nc.scalar.copy(yt[:, ot * 128:(ot + 1) * 128], ytr[:])
                cmax = ytpool.tile([128, 1], f32, name="cmax", tag="cmax")
                nc.vector.reduce_max(cmax[:], yt[:], axis=mybir.AxisListType.X)
                msk = maskpool.tile([128, O], f16, name="msk", tag="msk")
                nc.vector.tensor_scalar(msk[:], yt[:], cmax[:], patt[:, t:t + 1],
                                        op0=mybir.AluOpType.is_equal,
                                        op1=mybir.AluOpType.mult)
                masks.append(msk)

            # ---- hebb: lhsT=mask [126, o-tile], rhs = 9-tap xT block ----
            # xtblk[:, tap*128+c] = x16[c, t*126 + kh*64 + kw + p]; hebb_ps
            # columns are (ot, tap, c) so runs of taps with one ot merge into
            # single matmuls, split only at PSUM bank boundaries.
            for t in range(NT):
                last = (img == BL - 1 and t == NT - 1)
                xtblk = xtpool.tile([128, TAPS * C], f16, name="xtblk", tag="xt")
                for tap in range(TAPS):
                    q = t * TW + (tap // KS) * W + (tap % KS)
                    nc.sync.dma_start(xtblk[:, tap * C:(tap + 1) * C],
                                      x16[:, q:q + 128], transpose=True)
                for ot in range(OT):
                    base = ot * TAPS * C
                    # (c0, c1, ends_bank): stop only on the last chain per
                    # physical bank — ot0's third span shares bank 5 with
                    # ot1's first span, which is emitted later.
                    spans = ([(0, 512, True), (512, 1024, True),
                              (1024, 1152, False)] if ot == 0
                             else [(0, 384, True), (384, 896, True),
                                   (896, 1152, True)])
                    for c0, c1, ends_bank in spans:
                        nc.tensor.matmul(
                            hebb_ps[:, base + c0:base + c1],
                            masks[t][0:TW, ot * 128:(ot + 1) * 128],
                            xtblk[0:TW, c0:c1],
                            start=False, stop=(last and ends_bank))

        # ---- drain hebb ----
        hebb_sb = hdrainp.tile([128, HEBBC], f32, name="hebb_sb")
        nc.vector.tensor_copy(hebb_sb[:], hebb_ps[:])
        nc.sync.dma_start(hb_d[:], hebb_sb[:])


def kernel(x, weight, bias):
    x = np.ascontiguousarray(x, dtype=np.float32)
    weight = np.ascontiguousarray(weight, dtype=np.float32)
    bias = np.ascontiguousarray(bias, dtype=np.float32)

    if "nc" not in _CACHE:
        _CACHE["nc"] = build_nc()
    nc = _CACHE["nc"]

    ident, patt = _host_constants()
    shards = x.reshape(NCORES, BL, C, H, W)
    in_maps = [
        {"x": shards[i], "weight": weight, "bias": bias,
         "ident": ident, "patt": patt}
        for i in range(NCORES)
    ]
    res = run_bass_kernel_spmd(nc, in_maps, list(range(NCORES))).results

    y = np.concatenate([res[i]["y"] for i in range(NCORES)], axis=0)
    hsum = np.zeros((128, HEBBC), dtype=np.float32)
    for i in range(NCORES):
        hsum += res[i]["hebb"]
    # hebb[o_local, ot*1152 + tap*128 + c] -> delta_w[ot*128+o_local, c, kh, kw]
    h = hsum.reshape(128, OT, TAPS, C).transpose(1, 0, 3, 2)
    delta_w = (h.reshape(O, C, KS, KS) / DEN).astype(np.float32)
    return y, delta_w
